# revision 16
# baseline (speedup 1.0000x reference)
"""Trainium2 Bass kernel for nn_Attention (B=2, S=2048, D=1024, H=16).

Sharding: 8 cores = 2 batches x 4 head-groups (4 heads each), Megatron-style:
column-parallel QKV projections, local attention, row-parallel output
projection; host reduces the 4 partial outputs per batch and adds biases.

Per-core dataflow (all matmuls in float32r = full-rate TF32-like):
  stage 1: qT/kT = W @ x.T   (features on partitions, seq on free)
           v     = x @ W.T   (seq on partitions) with an appended ones column
  stage 2 (per head, per 1024-wide i-slice):
           sT[j,i] = k_j . q_i   -> exp on ACT -> e[j,i] (f32r)
           outT[d,i] accum over j of vplus[j,d]*e[j,i]  (row 64 = sum_j e = Z)
           normalize: outT[0:64]/Z via K=1 ones-matmul broadcast of 1/Z
  stage 3: finalT[f,i] = Wo_g.T stacked-heads matmul (partial, host-reduced)

Host folds the v-bias through attention (softmax rows sum to 1) into the
output bias: final_bias = bo + Wo @ bv.
"""

import numpy as np

import concourse.bass as bass
import concourse.mybir as mybir
import concourse.tile as tile
from concourse import bacc
from concourse.bass_utils import run_bass_kernel_spmd

B, S, D = 2, 2048, 1024
H, HD = 16, 64
G = 4              # head-groups == cores per batch
GH = H // G        # heads per core
F = GH * HD        # per-core projected features (256)
P = 128
KT = D // P        # 8 contraction tiles for the projections
NS = S // 512      # 4 seq tiles of 512
NC = S // P        # 16 seq chunks of 128
FR = mybir.dt.float32r
F32 = mybir.dt.float32
EXP = mybir.ActivationFunctionType.Exp

_CACHED = None


def _build():
    nc = bacc.Bacc("TRN2", target_bir_lowering=False, debug=False, num_devices=8)

    xq = nc.dram_tensor("xq", [D, S], FR, kind="ExternalInput").ap()
    xk = nc.dram_tensor("xk", [D, S], FR, kind="ExternalInput").ap()
    xv = nc.dram_tensor("xv", [D, S], FR, kind="ExternalInput").ap()
    wq = nc.dram_tensor("wq", [D, F], FR, kind="ExternalInput").ap()
    wk = nc.dram_tensor("wk", [D, F], FR, kind="ExternalInput").ap()
    wv = nc.dram_tensor("wv", [D, F], FR, kind="ExternalInput").ap()
    wo = nc.dram_tensor("wo", [F, D], FR, kind="ExternalInput").ap()
    bq = nc.dram_tensor("bq", [P, F // P], F32, kind="ExternalInput").ap()
    bk = nc.dram_tensor("bk", [P, F // P], F32, kind="ExternalInput").ap()
    ot = nc.dram_tensor("ot", [D, S], F32, kind="ExternalOutput").ap()

    xq_r = xq.rearrange("(ko p) s -> p ko s", p=P)
    xk_r = xk.rearrange("(ko p) s -> p ko s", p=P)
    xv_r = xv.rearrange("(ko p) s -> p ko s", p=P)
    wq_r = wq.rearrange("(ko p) f -> p ko f", p=P)
    wk_r = wk.rearrange("(ko p) f -> p ko f", p=P)
    wv_r = wv.rearrange("(ko p) f -> p ko f", p=P)
    wo_r = wo.rearrange("(ko p) f -> p ko f", p=P)
    ot_r = ot.rearrange("(fo p) s -> p fo s", p=P)

    with tile.TileContext(nc) as tc:
        with (
            tc.tile_pool(name="wpool", bufs=1) as wpool,
            tc.tile_pool(name="xpool", bufs=4) as xpool,
            tc.tile_pool(name="apool", bufs=1) as apool,
            tc.tile_pool(name="epool", bufs=3) as epool,
            tc.tile_pool(name="rpool", bufs=4) as rpool,
            tc.tile_pool(name="opool", bufs=3) as opool,
            tc.tile_pool(name="ps_s", bufs=2, space="PSUM") as ps_s,
            tc.tile_pool(name="ps_o", bufs=2, space="PSUM") as ps_o,
            tc.tile_pool(name="ps_m", bufs=2, space="PSUM") as ps_m,
        ):
            # ---- constants / weights (DMA-ordered by first use) ----
            wq_sb = wpool.tile([P, KT, F], FR)
            wk_sb = wpool.tile([P, KT, F], FR)
            wv_sb = wpool.tile([P, KT, F], FR)
            wo_sb = wpool.tile([P, F // P, D], FR)
            bq_sb = wpool.tile([P, F // P], F32)
            bk_sb = wpool.tile([P, F // P], F32)
            nc.sync.dma_start(wk_sb[:], wk_r)
            nc.sync.dma_start(bk_sb[:], bk)
            nc.sync.dma_start(wq_sb[:], wq_r)
            nc.sync.dma_start(bq_sb[:], bq)

            ones_f = wpool.tile([1, 64], F32)
            nc.vector.memset(ones_f[:], 1.0)
            ones_r = wpool.tile([1, 64], FR)
            nc.vector.tensor_copy(ones_r[:], ones_f[:])
            onesp_f = wpool.tile([P, 1], F32)
            nc.vector.memset(onesp_f[:], 1.0)
            onesp_r = wpool.tile([P, 1], FR)
            nc.vector.tensor_copy(onesp_r[:], onesp_f[:])

            # ---- persistent activations ----
            qT = [apool.tile([P, S], FR, name=f"qT{t}", tag=f"qT{t}") for t in range(2)]
            kT = [apool.tile([P, S], FR, name=f"kT{t}", tag=f"kT{t}") for t in range(2)]
            stack = [
                apool.tile([P, S], FR, name=f"stack{t}", tag=f"stack{t}")
                for t in range(2)
            ]
            v_sb = apool.tile([P, NC, GH, HD + 1], FR, name="v_sb")

            # ---- stage 1: k and v projections (then q) ----
            def proj_qk(dst, w_sb, b_sb, x_n, n):
                for t in range(2):
                    ps = ps_m.tile([P, 512], F32, tag="m")
                    for k in range(KT):
                        nc.tensor.matmul(
                            ps[:],
                            w_sb[:, k, t * P : (t + 1) * P],
                            x_n[:, k, :],
                            start=(k == 0),
                            stop=(k == KT - 1),
                        )
                    nc.vector.tensor_scalar_add(
                        dst[t][:, n * 512 : (n + 1) * 512], ps[:], b_sb[:, t : t + 1]
                    )

            def kproj(n):
                xk_n = xpool.tile([P, KT, 512], FR, tag="xin", name="xk_n")
                nc.sync.dma_start(xk_n[:], xk_r[:, :, n * 512 : (n + 1) * 512])
                proj_qk(kT, wk_sb, bk_sb, xk_n, n)

            def qproj(n):
                xq_n = xpool.tile([P, KT, 512], FR, tag="xin", name="xq_n")
                nc.sync.dma_start(xq_n[:], xq_r[:, :, n * 512 : (n + 1) * 512])
                proj_qk(qT, wq_sb, bq_sb, xq_n, n)

            def vproj(n):
                xv_n = xpool.tile([P, KT, 512], FR, tag="xin", name="xv_n")
                nc.sync.dma_start(xv_n[:], xv_r[:, :, n * 512 : (n + 1) * 512])
                for c in range(4):
                    psv = ps_m.tile([P, F], F32, tag="m")
                    for k in range(KT):
                        nc.tensor.matmul(
                            psv[:],
                            xv_n[:, k, c * P : (c + 1) * P],
                            wv_sb[:, k, :],
                            start=(k == 0),
                            stop=(k == KT - 1),
                        )
                    ch = n * 4 + c
                    nc.vector.tensor_copy(
                        v_sb[:, ch, :, 0:HD],
                        psv.rearrange("p (h e) -> p h e", e=HD),
                    )
                    nc.vector.tensor_copy(
                        v_sb[:, ch, :, HD : HD + 1],
                        onesp_r[:, 0:1, None].to_broadcast((P, GH, 1)),
                    )

            # order: everything attention needs first (k, first half of q),
            # v streamed just-in-time, rest of q and wo behind
            kproj(0)
            qproj(0)
            qproj(1)
            nc.sync.dma_start(wv_sb[:], wv_r)
            vproj(0)
            kproj(1)
            vproj(1)
            kproj(2)
            vproj(2)
            kproj(3)
            vproj(3)
            qproj(2)
            qproj(3)
            nc.sync.dma_start(wo_sb[:], wo_r)

            # ---- stage 2+3: attention (half-outer; output-projection units of
            # a finished i-half are interleaved between the next half's heads) ----
            def proj_unit(fb, n):
                pf = ps_m.tile([P, 512], F32, tag="m")
                for kk in range(F // P):
                    nc.tensor.matmul(
                        pf[:],
                        wo_sb[:, kk, fb * P : (fb + 1) * P],
                        stack[kk][:, n * 512 : (n + 1) * 512],
                        start=(kk == 0),
                        stop=(kk == F // P - 1),
                    )
                ob = opool.tile([P, 512], F32, tag="ob")
                nc.vector.tensor_copy(ob[:], pf[:])
                nc.sync.dma_start(ot_r[:, fb, n * 512 : (n + 1) * 512], ob[:])

            pending = []
            for half in range(2):
                i0 = half * 1024
                for h in range(GH):
                    t, r = h // 2, (h % 2) * 64
                    oacc = [
                        ps_o.tile([HD + 1, 512], F32, tag="oacc", name=f"oacc{_nn}")
                        for _nn in range(2)
                    ]
                    for j in range(NC):
                        ss = ps_s.tile([P, 1024], F32, tag="ss")
                        for nn in range(2):
                            nc.tensor.matmul(
                                ss[:, nn * 512 : (nn + 1) * 512],
                                kT[t][r : r + 64, j * P : (j + 1) * P],
                                qT[t][r : r + 64, i0 + nn * 512 : i0 + (nn + 1) * 512],
                                start=True,
                                stop=True,
                            )
                        eb = epool.tile([P, 1024], FR, tag="eb")
                        nc.scalar.activation(eb[:], ss[:], EXP)
                        for nn in range(2):
                            nc.tensor.matmul(
                                oacc[nn][:],
                                v_sb[:, j, h, :],
                                eb[:, nn * 512 : (nn + 1) * 512],
                                start=(j == 0),
                                stop=(j == NC - 1),
                            )
                        if j % 4 == 3 and pending:
                            proj_unit(*pending.pop(0))
                    for nn in range(2):
                        # copy psum out fast to free the accumulator slot, then
                        # normalize from SBUF off the critical path
                        ocp = rpool.tile([HD + 1, 512], F32, tag="ocp")
                        nc.vector.tensor_copy(ocp[:], oacc[nn][:])
                        rec = rpool.tile([1, 512], FR, tag="rec")
                        with nc.allow_low_precision(reason="f32r 1/Z, ~1e-4 rel"):
                            nc.vector.reciprocal(rec[:], ocp[HD : HD + 1, :])
                        bc = ps_m.tile([64, 512], F32, tag="m")
                        nc.tensor.matmul(bc[:], ones_r[:], rec[:], start=True, stop=True)
                        nc.vector.tensor_mul(
                            stack[t][r : r + 64, i0 + nn * 512 : i0 + (nn + 1) * 512],
                            ocp[0:HD, :],
                            bc[:],
                        )
                for nn in range(2):
                    for fb in range(D // P):
                        pending.append((fb, 2 * half + nn))
            for fb, n in pending:
                proj_unit(fb, n)

    nc.compile()
    return nc


def get_nc():
    global _CACHED
    if _CACHED is None:
        _CACHED = _build()
    return _CACHED


def make_in_maps(query, key, value, Wq, bq, Wk, bk, Wv, bv, Wo, bo):
    f32 = lambda a: np.ascontiguousarray(np.asarray(a, dtype=np.float32))
    in_maps = []
    for c in range(8):
        b, g = divmod(c, 4)
        gs = slice(F * g, F * (g + 1))
        in_maps.append(
            {
                "xq": f32(np.asarray(query)[b].T),
                "xk": f32(np.asarray(key)[b].T),
                "xv": f32(np.asarray(value)[b].T),
                "wq": f32(np.asarray(Wq)[gs, :].T),
                "wk": f32(np.asarray(Wk)[gs, :].T),
                "wv": f32(np.asarray(Wv)[gs, :].T),
                "wo": f32(np.asarray(Wo)[:, gs].T),
                "bq": f32(np.asarray(bq)[gs].reshape(F // P, P).T),
                "bk": f32(np.asarray(bk)[gs].reshape(F // P, P).T),
            }
        )
    return in_maps


def kernel(query, key, value, Wq, bq, Wk, bk, Wv, bv, Wo, bo):
    nc = get_nc()
    in_maps = make_in_maps(query, key, value, Wq, bq, Wk, bk, Wv, bv, Wo, bo)
    res = run_bass_kernel_spmd(nc, in_maps, core_ids=list(range(8)))
    bias_total = (
        np.asarray(bo, dtype=np.float64)
        + np.asarray(Wo, dtype=np.float64) @ np.asarray(bv, dtype=np.float64)
    ).astype(np.float32)
    outs = []
    for b in range(B):
        acc = np.zeros((D, S), np.float32)
        for g in range(G):
            acc += res.results[G * b + g]["ot"]
        outs.append(acc.T + bias_total[None, :])
    return np.stack(outs).astype(np.float32)


# revision 18
# speedup vs baseline: 88.4801x; 88.4801x over previous
"""Trainium2 Bass kernel for nn_Attention (B=2, S=2048, D=1024, H=16).

Sharding: 8 cores = 2 batches x 4 head-groups (4 heads each), Megatron-style:
column-parallel QKV projections, local attention, row-parallel output
projection; host reduces the 4 partial outputs per batch and adds biases.

Per-core dataflow (all matmuls in float32r = full-rate TF32-like):
  stage 1: qT/kT = W @ x.T   (features on partitions, seq on free)
           v     = x @ W.T   (seq on partitions) with an appended ones column
  stage 2 (per head, per 1024-wide i-slice):
           sT[j,i] = k_j . q_i   -> exp on ACT -> e[j,i] (f32r)
           outT[d,i] accum over j of vplus[j,d]*e[j,i]  (row 64 = sum_j e = Z)
           normalize: outT[0:64]/Z via K=1 ones-matmul broadcast of 1/Z
  stage 3: finalT[f,i] = Wo_g.T stacked-heads matmul (partial, host-reduced)

Host folds the v-bias through attention (softmax rows sum to 1) into the
output bias: final_bias = bo + Wo @ bv.
"""

import numpy as np

import concourse.bass as bass
import concourse.mybir as mybir
import concourse.tile as tile
from concourse import bacc
from concourse.bass_utils import run_bass_kernel_spmd

B, S, D = 2, 2048, 1024
H, HD = 16, 64
G = 4              # head-groups == cores per batch
GH = H // G        # heads per core
F = GH * HD        # per-core projected features (256)
P = 128
KT = D // P        # 8 contraction tiles for the projections
NS = S // 512      # 4 seq tiles of 512
NC = S // P        # 16 seq chunks of 128
FR = mybir.dt.float32r
F32 = mybir.dt.float32
EXP = mybir.ActivationFunctionType.Exp

_CACHED = None


def _build(reps=None):
    import contextlib

    nc = bacc.Bacc("TRN2", target_bir_lowering=False, debug=False, num_devices=8)

    xq = nc.dram_tensor("xq", [D, S], FR, kind="ExternalInput").ap()
    xk = nc.dram_tensor("xk", [D, S], FR, kind="ExternalInput").ap()
    xv = nc.dram_tensor("xv", [D, S], FR, kind="ExternalInput").ap()
    wq = nc.dram_tensor("wq", [D, F], FR, kind="ExternalInput").ap()
    wk = nc.dram_tensor("wk", [D, F], FR, kind="ExternalInput").ap()
    wv = nc.dram_tensor("wv", [D, F], FR, kind="ExternalInput").ap()
    wo = nc.dram_tensor("wo", [F, D], FR, kind="ExternalInput").ap()
    bq = nc.dram_tensor("bq", [P, F // P], F32, kind="ExternalInput").ap()
    bk = nc.dram_tensor("bk", [P, F // P], F32, kind="ExternalInput").ap()
    ot = nc.dram_tensor("ot", [D, S], F32, kind="ExternalOutput").ap()

    xq_r = xq.rearrange("(ko p) s -> p ko s", p=P)
    xk_r = xk.rearrange("(ko p) s -> p ko s", p=P)
    xv_r = xv.rearrange("(ko p) s -> p ko s", p=P)
    wq_r = wq.rearrange("(ko p) f -> p ko f", p=P)
    wk_r = wk.rearrange("(ko p) f -> p ko f", p=P)
    wv_r = wv.rearrange("(ko p) f -> p ko f", p=P)
    wo_r = wo.rearrange("(ko p) f -> p ko f", p=P)
    ot_r = ot.rearrange("(fo p) s -> p fo s", p=P)

    with tile.TileContext(nc) as tc:
        with (
            tc.tile_pool(name="wpool", bufs=1) as wpool,
            tc.tile_pool(name="xpool", bufs=4) as xpool,
            tc.tile_pool(name="apool", bufs=1) as apool,
            tc.tile_pool(name="epool", bufs=3) as epool,
            tc.tile_pool(name="rpool", bufs=4) as rpool,
            tc.tile_pool(name="opool", bufs=3) as opool,
            tc.tile_pool(name="ps_s", bufs=2, space="PSUM") as ps_s,
            tc.tile_pool(name="ps_o", bufs=2, space="PSUM") as ps_o,
            tc.tile_pool(name="ps_m", bufs=2, space="PSUM") as ps_m,
        ):
          with (
              tc.For_i(0, reps, 1, hint_engines=(mybir.EngineType.PE, mybir.EngineType.DVE, mybir.EngineType.Activation, mybir.EngineType.SP))
              if reps
              else contextlib.nullcontext()
          ):
            # ---- constants / weights (DMA-ordered by first use) ----
            wq_sb = wpool.tile([P, KT, F], FR)
            wk_sb = wpool.tile([P, KT, F], FR)
            wv_sb = wpool.tile([P, KT, F], FR)
            wo_sb = wpool.tile([P, F // P, D], FR)
            bq_sb = wpool.tile([P, F // P], F32)
            bk_sb = wpool.tile([P, F // P], F32)
            nc.sync.dma_start(wk_sb[:], wk_r)
            nc.sync.dma_start(bk_sb[:], bk)
            nc.sync.dma_start(wq_sb[:], wq_r)
            nc.sync.dma_start(bq_sb[:], bq)

            ones_f = wpool.tile([1, 64], F32)
            nc.vector.memset(ones_f[:], 1.0)
            ones_r = wpool.tile([1, 64], FR)
            nc.vector.tensor_copy(ones_r[:], ones_f[:])
            onesp_f = wpool.tile([P, 1], F32)
            nc.vector.memset(onesp_f[:], 1.0)
            onesp_r = wpool.tile([P, 1], FR)
            nc.vector.tensor_copy(onesp_r[:], onesp_f[:])

            # ---- persistent activations ----
            qT = [apool.tile([P, S], FR, name=f"qT{t}", tag=f"qT{t}") for t in range(2)]
            kT = [apool.tile([P, S], FR, name=f"kT{t}", tag=f"kT{t}") for t in range(2)]
            stack = [
                apool.tile([P, S], FR, name=f"stack{t}", tag=f"stack{t}")
                for t in range(2)
            ]
            v_sb = apool.tile([P, NC, GH, HD + 1], FR, name="v_sb")

            # ---- stage 1: k and v projections (then q) ----
            def proj_qk(dst, w_sb, b_sb, x_n, n):
                for t in range(2):
                    ps = ps_m.tile([P, 512], F32, tag="m")
                    for k in range(KT):
                        nc.tensor.matmul(
                            ps[:],
                            w_sb[:, k, t * P : (t + 1) * P],
                            x_n[:, k, :],
                            start=(k == 0),
                            stop=(k == KT - 1),
                        )
                    nc.vector.tensor_scalar_add(
                        dst[t][:, n * 512 : (n + 1) * 512], ps[:], b_sb[:, t : t + 1]
                    )

            def kproj(n):
                xk_n = xpool.tile([P, KT, 512], FR, tag="xin", name="xk_n")
                nc.sync.dma_start(xk_n[:], xk_r[:, :, n * 512 : (n + 1) * 512])
                proj_qk(kT, wk_sb, bk_sb, xk_n, n)

            def qproj(n):
                xq_n = xpool.tile([P, KT, 512], FR, tag="xin", name="xq_n")
                nc.sync.dma_start(xq_n[:], xq_r[:, :, n * 512 : (n + 1) * 512])
                proj_qk(qT, wq_sb, bq_sb, xq_n, n)

            def vproj(n):
                xv_n = xpool.tile([P, KT, 512], FR, tag="xin", name="xv_n")
                nc.sync.dma_start(xv_n[:], xv_r[:, :, n * 512 : (n + 1) * 512])
                for c in range(4):
                    psv = ps_m.tile([P, F], F32, tag="m")
                    for k in range(KT):
                        nc.tensor.matmul(
                            psv[:],
                            xv_n[:, k, c * P : (c + 1) * P],
                            wv_sb[:, k, :],
                            start=(k == 0),
                            stop=(k == KT - 1),
                        )
                    ch = n * 4 + c
                    nc.vector.tensor_copy(
                        v_sb[:, ch, :, 0:HD],
                        psv.rearrange("p (h e) -> p h e", e=HD),
                    )
                    nc.vector.tensor_copy(
                        v_sb[:, ch, :, HD : HD + 1],
                        onesp_r[:, 0:1, None].to_broadcast((P, GH, 1)),
                    )

            # order: everything attention needs first (k, first half of q),
            # v streamed just-in-time, rest of q and wo behind
            kproj(0)
            qproj(0)
            qproj(1)
            nc.sync.dma_start(wv_sb[:], wv_r)
            vproj(0)
            kproj(1)
            vproj(1)
            kproj(2)
            vproj(2)
            kproj(3)
            vproj(3)
            qproj(2)
            qproj(3)
            nc.sync.dma_start(wo_sb[:], wo_r)

            # ---- stage 2+3: attention (half-outer; output-projection units of
            # a finished i-half are interleaved between the next half's heads) ----
            def proj_unit(fb, n):
                pf = ps_m.tile([P, 512], F32, tag="m")
                for kk in range(F // P):
                    nc.tensor.matmul(
                        pf[:],
                        wo_sb[:, kk, fb * P : (fb + 1) * P],
                        stack[kk][:, n * 512 : (n + 1) * 512],
                        start=(kk == 0),
                        stop=(kk == F // P - 1),
                    )
                ob = opool.tile([P, 512], F32, tag="ob")
                nc.vector.tensor_copy(ob[:], pf[:])
                nc.sync.dma_start(ot_r[:, fb, n * 512 : (n + 1) * 512], ob[:])

            pending = []
            for half in range(2):
                i0 = half * 1024
                for h in range(GH):
                    t, r = h // 2, (h % 2) * 64
                    oacc = [
                        ps_o.tile([HD + 1, 512], F32, tag="oacc", name=f"oacc{_nn}")
                        for _nn in range(2)
                    ]
                    for j in range(NC):
                        ss = ps_s.tile([P, 1024], F32, tag="ss")
                        for nn in range(2):
                            nc.tensor.matmul(
                                ss[:, nn * 512 : (nn + 1) * 512],
                                kT[t][r : r + 64, j * P : (j + 1) * P],
                                qT[t][r : r + 64, i0 + nn * 512 : i0 + (nn + 1) * 512],
                                start=True,
                                stop=True,
                            )
                        eb = epool.tile([P, 1024], FR, tag="eb")
                        nc.scalar.activation(eb[:], ss[:], EXP)
                        for nn in range(2):
                            nc.tensor.matmul(
                                oacc[nn][:],
                                v_sb[:, j, h, :],
                                eb[:, nn * 512 : (nn + 1) * 512],
                                start=(j == 0),
                                stop=(j == NC - 1),
                            )
                        if j % 4 == 3 and pending:
                            proj_unit(*pending.pop(0))
                    for nn in range(2):
                        # copy psum out fast to free the accumulator slot, then
                        # normalize from SBUF off the critical path
                        ocp = rpool.tile([HD + 1, 512], F32, tag="ocp")
                        nc.vector.tensor_copy(ocp[:], oacc[nn][:])
                        rec = rpool.tile([1, 512], FR, tag="rec")
                        with nc.allow_low_precision(reason="f32r 1/Z, ~1e-4 rel"):
                            nc.vector.reciprocal(rec[:], ocp[HD : HD + 1, :])
                        bc = ps_m.tile([64, 512], F32, tag="m")
                        nc.tensor.matmul(bc[:], ones_r[:], rec[:], start=True, stop=True)
                        nc.vector.tensor_mul(
                            stack[t][r : r + 64, i0 + nn * 512 : i0 + (nn + 1) * 512],
                            ocp[0:HD, :],
                            bc[:],
                        )
                for nn in range(2):
                    for fb in range(D // P):
                        pending.append((fb, 2 * half + nn))
            for fb, n in pending:
                proj_unit(fb, n)

    nc.compile()
    return nc


def get_nc():
    global _CACHED
    if _CACHED is None:
        _CACHED = _build()
    return _CACHED


def make_in_maps(query, key, value, Wq, bq, Wk, bk, Wv, bv, Wo, bo):
    f32 = lambda a: np.ascontiguousarray(np.asarray(a, dtype=np.float32))
    in_maps = []
    for c in range(8):
        b, g = divmod(c, 4)
        gs = slice(F * g, F * (g + 1))
        in_maps.append(
            {
                "xq": f32(np.asarray(query)[b].T),
                "xk": f32(np.asarray(key)[b].T),
                "xv": f32(np.asarray(value)[b].T),
                "wq": f32(np.asarray(Wq)[gs, :].T),
                "wk": f32(np.asarray(Wk)[gs, :].T),
                "wv": f32(np.asarray(Wv)[gs, :].T),
                "wo": f32(np.asarray(Wo)[:, gs].T),
                "bq": f32(np.asarray(bq)[gs].reshape(F // P, P).T),
                "bk": f32(np.asarray(bk)[gs].reshape(F // P, P).T),
            }
        )
    return in_maps


def kernel(query, key, value, Wq, bq, Wk, bk, Wv, bv, Wo, bo):
    nc = get_nc()
    in_maps = make_in_maps(query, key, value, Wq, bq, Wk, bk, Wv, bv, Wo, bo)
    res = run_bass_kernel_spmd(nc, in_maps, core_ids=list(range(8)))
    bias_total = (
        np.asarray(bo, dtype=np.float64)
        + np.asarray(Wo, dtype=np.float64) @ np.asarray(bv, dtype=np.float64)
    ).astype(np.float32)
    outs = []
    for b in range(B):
        acc = np.zeros((D, S), np.float32)
        for g in range(G):
            acc += res.results[G * b + g]["ot"]
        outs.append(acc.T + bias_total[None, :])
    return np.stack(outs).astype(np.float32)


# revision 22
# speedup vs baseline: 99.1861x; 1.1210x over previous
"""Trainium2 Bass kernel for nn_Attention (B=2, S=2048, D=1024, H=16).

Sharding: 8 cores = 2 batches x 4 head-groups (4 heads each), Megatron-style:
column-parallel QKV projections, local attention, row-parallel output
projection; host reduces the 4 partial outputs per batch and adds biases.

Per-core dataflow (all matmuls in float32r = full-rate TF32-like):
  stage 1: qT/kT = W @ x.T   (features on partitions, seq on free)
           v     = x @ W.T   (seq on partitions) with an appended ones column
  stage 2 (per head, per 1024-wide i-slice):
           sT[j,i] = k_j . q_i   -> exp on ACT -> e[j,i] (f32r)
           outT[d,i] accum over j of vplus[j,d]*e[j,i]  (row 64 = sum_j e = Z)
           normalize: outT[0:64]/Z via K=1 ones-matmul broadcast of 1/Z
  stage 3: finalT[f,i] = Wo_g.T stacked-heads matmul (partial, host-reduced)

Host folds the v-bias through attention (softmax rows sum to 1) into the
output bias: final_bias = bo + Wo @ bv.
"""

import numpy as np

import concourse.bass as bass
import concourse.mybir as mybir
import concourse.tile as tile
from concourse import bacc
from concourse.bass_utils import run_bass_kernel_spmd

B, S, D = 2, 2048, 1024
H, HD = 16, 64
G = 4              # head-groups == cores per batch
GH = H // G        # heads per core
F = GH * HD        # per-core projected features (256)
P = 128
KT = D // P        # 8 contraction tiles for the projections
NS = S // 512      # 4 seq tiles of 512
NC = S // P        # 16 seq chunks of 128
FR = mybir.dt.float32r
F32 = mybir.dt.float32
EXP = mybir.ActivationFunctionType.Exp

_CACHED = None


def _build(reps=None):
    import contextlib

    nc = bacc.Bacc("TRN2", target_bir_lowering=False, debug=False, num_devices=8)

    xq = nc.dram_tensor("xq", [D, S], FR, kind="ExternalInput").ap()
    xk = nc.dram_tensor("xk", [D, S], FR, kind="ExternalInput").ap()
    xv = nc.dram_tensor("xv", [D, S], FR, kind="ExternalInput").ap()
    wq = nc.dram_tensor("wq", [D, F], FR, kind="ExternalInput").ap()
    wk = nc.dram_tensor("wk", [D, F], FR, kind="ExternalInput").ap()
    wv = nc.dram_tensor("wv", [D, F], FR, kind="ExternalInput").ap()
    wo = nc.dram_tensor("wo", [F, D], FR, kind="ExternalInput").ap()
    bq = nc.dram_tensor("bq", [P, F // P], F32, kind="ExternalInput").ap()
    bk = nc.dram_tensor("bk", [P, F // P], F32, kind="ExternalInput").ap()
    ot = nc.dram_tensor("ot", [D, S], F32, kind="ExternalOutput").ap()

    xq_r = xq.rearrange("(ko p) s -> p ko s", p=P)
    xk_r = xk.rearrange("(ko p) s -> p ko s", p=P)
    xv_r = xv.rearrange("(ko p) s -> p ko s", p=P)
    wq_r = wq.rearrange("(ko p) f -> p ko f", p=P)
    wk_r = wk.rearrange("(ko p) f -> p ko f", p=P)
    wv_r = wv.rearrange("(ko p) f -> p ko f", p=P)
    wo_r = wo.rearrange("(ko p) f -> p ko f", p=P)
    ot_r = ot.rearrange("(fo p) s -> p fo s", p=P)

    with tile.TileContext(nc) as tc:
        with (
            tc.tile_pool(name="wpool", bufs=1) as wpool,
            tc.tile_pool(name="xpool", bufs=2) as xpool,
            tc.tile_pool(name="apool", bufs=1) as apool,
            tc.tile_pool(name="epool", bufs=3) as epool,
            tc.tile_pool(name="rpool", bufs=3) as rpool,
            tc.tile_pool(name="opool", bufs=2) as opool,
            tc.tile_pool(name="ps_s", bufs=2, space="PSUM") as ps_s,
            tc.tile_pool(name="ps_o", bufs=2, space="PSUM") as ps_o,
            tc.tile_pool(name="ps_m", bufs=2, space="PSUM") as ps_m,
        ):
          with (
              tc.For_i(0, reps, 1, hint_engines=(mybir.EngineType.PE, mybir.EngineType.DVE, mybir.EngineType.Activation, mybir.EngineType.SP))
              if reps
              else contextlib.nullcontext()
          ):
            # ---- constants / weights (DMA-ordered by first use) ----
            wq_sb = wpool.tile([P, KT, F], FR)
            wk_sb = wpool.tile([P, KT, F], FR)
            wv_sb = wpool.tile([P, KT, F], FR)
            wo_sb = wpool.tile([P, F // P, D], FR)
            bq_sb = wpool.tile([P, F // P], F32)
            bk_sb = wpool.tile([P, F // P], F32)
            nc.sync.dma_start(wk_sb[:], wk_r)
            nc.sync.dma_start(bk_sb[:], bk)
            nc.sync.dma_start(wq_sb[:], wq_r)
            nc.sync.dma_start(bq_sb[:], bq)

            ones_f = wpool.tile([1, 64], F32)
            nc.vector.memset(ones_f[:], 1.0)
            ones_r = wpool.tile([1, 64], FR)
            nc.vector.tensor_copy(ones_r[:], ones_f[:])
            onesp_f = wpool.tile([P, 1], F32)
            nc.vector.memset(onesp_f[:], 1.0)
            onesp_r = wpool.tile([P, 1], FR)
            nc.vector.tensor_copy(onesp_r[:], onesp_f[:])

            # ---- persistent activations ----
            # per-head q/k tiles with rows duplicated into both partition
            # halves, so a head's two i-halves can run as one row-packed
            # matmul pair (tile_position (0,0) / (64,0))
            qTd = [
                apool.tile([P, S], FR, name=f"qTd{h}", tag=f"qTd{h}") for h in range(GH)
            ]
            kTd = [
                apool.tile([P, S], FR, name=f"kTd{h}", tag=f"kTd{h}") for h in range(GH)
            ]
            stack = [
                apool.tile([P, S], FR, name=f"stack{t}", tag=f"stack{t}")
                for t in range(2)
            ]
            v_sb = apool.tile([P, NC, GH, HD + 1], FR, name="v_sb")

            # ---- stage 1: k and v projections (then q) ----
            def proj_qk(dst, w_sb, b_sb, x_n, n):
                for t in range(2):
                    ps = ps_m.tile([P, 512], F32, tag="m")
                    for k in range(KT):
                        nc.tensor.matmul(
                            ps[:],
                            w_sb[:, k, t * P : (t + 1) * P],
                            x_n[:, k, :],
                            start=(k == 0),
                            stop=(k == KT - 1),
                        )
                    for hh in range(2):
                        h = 2 * t + hh
                        r = hh * 64
                        for dup in range(2):
                            nc.vector.tensor_scalar_add(
                                dst[h][dup * 64 : dup * 64 + 64, n * 512 : (n + 1) * 512],
                                ps[r : r + 64, :],
                                b_sb[r : r + 64, t : t + 1],
                            )

            def kproj(n):
                xk_n = xpool.tile([P, KT, 512], FR, tag="xin", name="xk_n")
                nc.sync.dma_start(xk_n[:], xk_r[:, :, n * 512 : (n + 1) * 512])
                proj_qk(kTd, wk_sb, bk_sb, xk_n, n)

            def qproj(n):
                xq_n = xpool.tile([P, KT, 512], FR, tag="xin", name="xq_n")
                nc.sync.dma_start(xq_n[:], xq_r[:, :, n * 512 : (n + 1) * 512])
                proj_qk(qTd, wq_sb, bq_sb, xq_n, n)

            def vproj(n):
                xv_n = xpool.tile([P, KT, 512], FR, tag="xin", name="xv_n")
                nc.sync.dma_start(xv_n[:], xv_r[:, :, n * 512 : (n + 1) * 512])
                for c in range(4):
                    psv = ps_m.tile([P, F], F32, tag="m")
                    for k in range(KT):
                        nc.tensor.matmul(
                            psv[:],
                            xv_n[:, k, c * P : (c + 1) * P],
                            wv_sb[:, k, :],
                            start=(k == 0),
                            stop=(k == KT - 1),
                        )
                    ch = n * 4 + c
                    nc.vector.tensor_copy(
                        v_sb[:, ch, :, 0:HD],
                        psv.rearrange("p (h e) -> p h e", e=HD),
                    )
                    nc.vector.tensor_copy(
                        v_sb[:, ch, :, HD : HD + 1],
                        onesp_r[:, 0:1, None].to_broadcast((P, GH, 1)),
                    )

            # order: everything attention needs first (k, first half of q),
            # v streamed just-in-time, rest of q and wo behind
            kproj(0)
            qproj(0)
            qproj(1)
            nc.sync.dma_start(wv_sb[:], wv_r)
            vproj(0)
            kproj(1)
            vproj(1)
            kproj(2)
            vproj(2)
            kproj(3)
            vproj(3)
            qproj(2)
            qproj(3)
            nc.sync.dma_start(wo_sb[:], wo_r)

            # ---- stage 2+3: attention (half-outer; output-projection units of
            # a finished i-half are interleaved between the next half's heads) ----
            def proj_unit(fb, n):
                pf = ps_m.tile([P, 512], F32, tag="m")
                for kk in range(F // P):
                    nc.tensor.matmul(
                        pf[:],
                        wo_sb[:, kk, fb * P : (fb + 1) * P],
                        stack[kk][:, n * 512 : (n + 1) * 512],
                        start=(kk == 0),
                        stop=(kk == F // P - 1),
                    )
                ob = opool.tile([P, 512], F32, tag="ob")
                nc.vector.tensor_copy(ob[:], pf[:])
                nc.sync.dma_start(ot_r[:, fb, n * 512 : (n + 1) * 512], ob[:])

            pending = []
            for half in range(2):
                i0 = half * 1024
                for h in range(GH):
                    t, r = h // 2, (h % 2) * 64
                    oacc = [
                        ps_o.tile([HD + 1, 512], F32, tag="oacc", name=f"oacc{_nn}")
                        for _nn in range(2)
                    ]
                    for j in range(NC):
                        ss = ps_s.tile([P, 1024], F32, tag="ss")
                        for nn in range(2):
                            rb = nn * 64
                            nc.tensor.matmul(
                                ss[:, nn * 512 : (nn + 1) * 512],
                                kTd[h][rb : rb + 64, j * P : (j + 1) * P],
                                qTd[h][rb : rb + 64, i0 + nn * 512 : i0 + (nn + 1) * 512],
                                start=True,
                                stop=True,
                                tile_position=(rb, 0),
                            )
                        eb = epool.tile([P, 1024], FR, tag="eb")
                        nc.scalar.activation(eb[:], ss[:], EXP)
                        for nn in range(2):
                            nc.tensor.matmul(
                                oacc[nn][:],
                                v_sb[:, j, h, :],
                                eb[:, nn * 512 : (nn + 1) * 512],
                                start=(j == 0),
                                stop=(j == NC - 1),
                            )
                        if j % 4 == 3 and pending:
                            proj_unit(*pending.pop(0))
                    for nn in range(2):
                        # copy psum out fast to free the accumulator slot, then
                        # normalize from SBUF off the critical path
                        ocp = rpool.tile([HD + 1, 512], F32, tag="ocp")
                        nc.vector.tensor_copy(ocp[:], oacc[nn][:])
                        rec = rpool.tile([1, 512], FR, tag="rec")
                        with nc.allow_low_precision(reason="f32r 1/Z, ~1e-4 rel"):
                            nc.vector.reciprocal(rec[:], ocp[HD : HD + 1, :])
                        bc = ps_m.tile([64, 512], F32, tag="m")
                        nc.tensor.matmul(bc[:], ones_r[:], rec[:], start=True, stop=True)
                        nc.vector.tensor_mul(
                            stack[t][r : r + 64, i0 + nn * 512 : i0 + (nn + 1) * 512],
                            ocp[0:HD, :],
                            bc[:],
                        )
                for nn in range(2):
                    for fb in range(D // P):
                        pending.append((fb, 2 * half + nn))
            for fb, n in pending:
                proj_unit(fb, n)

    nc.compile()
    return nc


def get_nc():
    global _CACHED
    if _CACHED is None:
        _CACHED = _build()
    return _CACHED


def make_in_maps(query, key, value, Wq, bq, Wk, bk, Wv, bv, Wo, bo):
    f32 = lambda a: np.ascontiguousarray(np.asarray(a, dtype=np.float32))
    in_maps = []
    for c in range(8):
        b, g = divmod(c, 4)
        gs = slice(F * g, F * (g + 1))
        in_maps.append(
            {
                "xq": f32(np.asarray(query)[b].T),
                "xk": f32(np.asarray(key)[b].T),
                "xv": f32(np.asarray(value)[b].T),
                "wq": f32(np.asarray(Wq)[gs, :].T),
                "wk": f32(np.asarray(Wk)[gs, :].T),
                "wv": f32(np.asarray(Wv)[gs, :].T),
                "wo": f32(np.asarray(Wo)[:, gs].T),
                "bq": f32(np.asarray(bq)[gs].reshape(F // P, P).T),
                "bk": f32(np.asarray(bk)[gs].reshape(F // P, P).T),
            }
        )
    return in_maps


def kernel(query, key, value, Wq, bq, Wk, bk, Wv, bv, Wo, bo):
    nc = get_nc()
    in_maps = make_in_maps(query, key, value, Wq, bq, Wk, bk, Wv, bv, Wo, bo)
    res = run_bass_kernel_spmd(nc, in_maps, core_ids=list(range(8)))
    bias_total = (
        np.asarray(bo, dtype=np.float64)
        + np.asarray(Wo, dtype=np.float64) @ np.asarray(bv, dtype=np.float64)
    ).astype(np.float32)
    outs = []
    for b in range(B):
        acc = np.zeros((D, S), np.float32)
        for g in range(G):
            acc += res.results[G * b + g]["ot"]
        outs.append(acc.T + bias_total[None, :])
    return np.stack(outs).astype(np.float32)


# revision 34
# speedup vs baseline: 99.6920x; 1.0051x over previous
"""Trainium2 Bass kernel for nn_Attention (B=2, S=2048, D=1024, H=16).

Sharding: 8 cores = 2 batches x 4 head-groups (4 heads each), Megatron-style:
column-parallel QKV projections, local attention, row-parallel output
projection; host reduces the 4 partial outputs per batch and adds biases.

Per-core dataflow (all matmuls in float32r = full-rate TF32-like):
  stage 1: qT/kT = W @ x.T   (features on partitions, seq on free)
           v     = x @ W.T   (seq on partitions) with an appended ones column
  stage 2 (per head, per 1024-wide i-slice):
           sT[j,i] = k_j . q_i   -> exp on ACT -> e[j,i] (f32r)
           outT[d,i] accum over j of vplus[j,d]*e[j,i]  (row 64 = sum_j e = Z)
           normalize: outT[0:64]/Z via K=1 ones-matmul broadcast of 1/Z
  stage 3: finalT[f,i] = Wo_g.T stacked-heads matmul (partial, host-reduced)

Host folds the v-bias through attention (softmax rows sum to 1) into the
output bias: final_bias = bo + Wo @ bv.
"""

import numpy as np

import concourse.bass as bass
import concourse.mybir as mybir
import concourse.tile as tile
from concourse import bacc
from concourse.bass_utils import run_bass_kernel_spmd

B, S, D = 2, 2048, 1024
H, HD = 16, 64
G = 4              # head-groups == cores per batch
GH = H // G        # heads per core
F = GH * HD        # per-core projected features (256)
P = 128
KT = D // P        # 8 contraction tiles for the projections
NS = S // 512      # 4 seq tiles of 512
NC = S // P        # 16 seq chunks of 128
FR = mybir.dt.float32r
F32 = mybir.dt.float32
EXP = mybir.ActivationFunctionType.Exp

_CACHED = None


def _build(reps=None):
    import contextlib

    nc = bacc.Bacc("TRN2", target_bir_lowering=False, debug=False, num_devices=8)

    xq = nc.dram_tensor("xq", [D, S], FR, kind="ExternalInput").ap()
    xk = nc.dram_tensor("xk", [D, S], FR, kind="ExternalInput").ap()
    xv = nc.dram_tensor("xv", [D, S], FR, kind="ExternalInput").ap()
    wq = nc.dram_tensor("wq", [D, F], FR, kind="ExternalInput").ap()
    wk = nc.dram_tensor("wk", [D, F], FR, kind="ExternalInput").ap()
    wv = nc.dram_tensor("wv", [D, F], FR, kind="ExternalInput").ap()
    wo = nc.dram_tensor("wo", [F, D], FR, kind="ExternalInput").ap()
    bq = nc.dram_tensor("bq", [P, F // P], F32, kind="ExternalInput").ap()
    bk = nc.dram_tensor("bk", [P, F // P], F32, kind="ExternalInput").ap()
    ot = nc.dram_tensor("ot", [D, S], F32, kind="ExternalOutput").ap()

    xq_r = xq.rearrange("(ko p) s -> p ko s", p=P)
    xk_r = xk.rearrange("(ko p) s -> p ko s", p=P)
    xv_r = xv.rearrange("(ko p) s -> p ko s", p=P)
    wq_r = wq.rearrange("(ko p) f -> p ko f", p=P)
    wk_r = wk.rearrange("(ko p) f -> p ko f", p=P)
    wv_r = wv.rearrange("(ko p) f -> p ko f", p=P)
    wo_r = wo.rearrange("(ko p) f -> p ko f", p=P)
    ot_r = ot.rearrange("(fo p) s -> p fo s", p=P)

    with tile.TileContext(nc) as tc:
        with (
            tc.tile_pool(name="wpool", bufs=1) as wpool,
            tc.tile_pool(name="xpool", bufs=2) as xpool,
            tc.tile_pool(name="apool", bufs=1) as apool,
            tc.tile_pool(name="epool", bufs=4) as epool,
            tc.tile_pool(name="rpool", bufs=3) as rpool,
            tc.tile_pool(name="opool", bufs=2) as opool,
            tc.tile_pool(name="ps_s", bufs=2, space="PSUM") as ps_s,
            tc.tile_pool(name="ps_o", bufs=2, space="PSUM") as ps_o,
            tc.tile_pool(name="ps_m", bufs=2, space="PSUM") as ps_m,
        ):
          with (
              tc.For_i(0, reps, 1, hint_engines=(mybir.EngineType.PE, mybir.EngineType.DVE, mybir.EngineType.Activation, mybir.EngineType.SP))
              if reps
              else contextlib.nullcontext()
          ):
            # ---- constants / weights (DMA-ordered by first use) ----
            wq_sb = wpool.tile([P, KT, F], FR)
            wk_sb = wpool.tile([P, KT, F], FR)
            wv_sb = wpool.tile([P, KT, F], FR)
            wo_sb = wpool.tile([P, F // P, D], FR)
            bq_sb = wpool.tile([P, F // P], F32)
            bk_sb = wpool.tile([P, F // P], F32)
            nc.sync.dma_start(wk_sb[:], wk_r)
            nc.sync.dma_start(bk_sb[:], bk)
            nc.sync.dma_start(wq_sb[:], wq_r)
            nc.sync.dma_start(bq_sb[:], bq)

            ones_f = wpool.tile([1, 64], F32)
            nc.vector.memset(ones_f[:], 1.0)
            ones_r = wpool.tile([1, 64], FR)
            nc.vector.tensor_copy(ones_r[:], ones_f[:])
            onesp_f = wpool.tile([P, 1], F32)
            nc.vector.memset(onesp_f[:], 1.0)
            onesp_r = wpool.tile([P, 1], FR)
            nc.vector.tensor_copy(onesp_r[:], onesp_f[:])

            # ---- persistent activations ----
            # per-head q/k tiles with rows duplicated into both partition
            # halves, so a head's two i-halves can run as one row-packed
            # matmul pair (tile_position (0,0) / (64,0))
            qTd = [
                apool.tile([P, S], FR, name=f"qTd{h}", tag=f"qTd{h}") for h in range(GH)
            ]
            kTd = [
                apool.tile([P, S], FR, name=f"kTd{h}", tag=f"kTd{h}") for h in range(GH)
            ]
            stack = [
                apool.tile([P, S], FR, name=f"stack{t}", tag=f"stack{t}")
                for t in range(2)
            ]
            v_sb = apool.tile([P, NC, GH, HD + 1], FR, name="v_sb")

            # ---- stage 1: k and v projections (then q) ----
            def proj_qk_t(dst, w_sb, b_sb, x_n, n, t):
                ps = ps_m.tile([P, 512], F32, tag="m", name="ps")
                for k in range(KT):
                    nc.tensor.matmul(
                        ps[:],
                        w_sb[:, k, t * P : (t + 1) * P],
                        x_n[:, k, :],
                        start=(k == 0),
                        stop=(k == KT - 1),
                    )
                for hh in range(2):
                    h = 2 * t + hh
                    r = hh * 64
                    for dup in range(2):
                        nc.vector.tensor_scalar_add(
                            dst[h][dup * 64 : dup * 64 + 64, n * 512 : (n + 1) * 512],
                            ps[r : r + 64, :],
                            b_sb[r : r + 64, t : t + 1],
                        )

            def proj_qk(dst, w_sb, b_sb, x_n, n):
                for t in range(2):
                    proj_qk_t(dst, w_sb, b_sb, x_n, n, t)

            def kproj(n, ts=(0, 1)):
                xk_n = xpool.tile([P, KT, 512], FR, tag="xin", name="xk_n")
                nc.sync.dma_start(xk_n[:], xk_r[:, :, n * 512 : (n + 1) * 512])
                for t in ts:
                    proj_qk_t(kTd, wk_sb, bk_sb, xk_n, n, t)

            def qproj(n, ts=(0, 1)):
                xq_n = xpool.tile([P, KT, 512], FR, tag="xin", name="xq_n")
                nc.sync.dma_start(xq_n[:], xq_r[:, :, n * 512 : (n + 1) * 512])
                for t in ts:
                    proj_qk_t(qTd, wq_sb, bq_sb, xq_n, n, t)

            def vproj(n):
                xv_n = xpool.tile([P, KT, 512], FR, tag="xin", name="xv_n")
                nc.sync.dma_start(xv_n[:], xv_r[:, :, n * 512 : (n + 1) * 512])
                for c in range(4):
                    psv = ps_m.tile([P, F], F32, tag="m")
                    for k in range(KT):
                        nc.tensor.matmul(
                            psv[:],
                            xv_n[:, k, c * P : (c + 1) * P],
                            wv_sb[:, k, :],
                            start=(k == 0),
                            stop=(k == KT - 1),
                        )
                    ch = n * 4 + c
                    nc.vector.tensor_copy(
                        v_sb[:, ch, :, 0:HD],
                        psv.rearrange("p (h e) -> p h e", e=HD),
                    )
                    nc.vector.tensor_copy(
                        v_sb[:, ch, :, HD : HD + 1],
                        onesp_r[:, 0:1, None].to_broadcast((P, GH, 1)),
                    )

            # order: the minimum attention-head-0/1 needs runs up front
            # (t=0 halves of k and first-half q, all of v); the rest is
            # deferred into the attention loop's interleave slots where the
            # PE would otherwise idle while ACT grinds exps
            kproj(0)
            qproj(0)
            qproj(1)
            nc.sync.dma_start(wv_sb[:], wv_r)
            vproj(0)
            kproj(1)
            vproj(1)
            kproj(2)
            vproj(2)
            kproj(3)
            vproj(3)
            nc.sync.dma_start(wo_sb[:], wo_r)

            # ---- stage 2+3: attention (half-outer; output-projection units of
            # a finished i-half are interleaved between the next half's heads) ----
            def proj_unit(fb, n):
                pf = ps_m.tile([P, 512], F32, tag="m")
                for kk in range(F // P):
                    nc.tensor.matmul(
                        pf[:],
                        wo_sb[:, kk, fb * P : (fb + 1) * P],
                        stack[kk][:, n * 512 : (n + 1) * 512],
                        start=(kk == 0),
                        stop=(kk == F // P - 1),
                    )
                ob = opool.tile([P, 512], F32, tag="ob")
                nc.vector.tensor_copy(ob[:], pf[:])
                nc.sync.dma_start(ot_r[:, fb, n * 512 : (n + 1) * 512], ob[:])

            pending = [lambda n=n: qproj(n) for n in (2, 3)]
            for half in range(2):
                i0 = half * 1024
                for h in range(GH):
                    t, r = h // 2, (h % 2) * 64
                    oacc = [
                        ps_o.tile([HD + 1, 512], F32, tag="oacc", name=f"oacc{_nn}")
                        for _nn in range(2)
                    ]
                    for j in range(NC):
                        ss = ps_s.tile([P, 1024], F32, tag="ss")
                        for nn in range(2):
                            rb = nn * 64
                            nc.tensor.matmul(
                                ss[:, nn * 512 : (nn + 1) * 512],
                                kTd[h][rb : rb + 64, j * P : (j + 1) * P],
                                qTd[h][rb : rb + 64, i0 + nn * 512 : i0 + (nn + 1) * 512],
                                start=True,
                                stop=True,
                                tile_position=(rb, 0),
                            )
                        eb = epool.tile([P, 1024], FR, tag="eb")
                        nc.scalar.activation(eb[:], ss[:], EXP)
                        for nn in range(2):
                            nc.tensor.matmul(
                                oacc[nn][:],
                                v_sb[:, j, h, :],
                                eb[:, nn * 512 : (nn + 1) * 512],
                                start=(j == 0),
                                stop=(j == NC - 1),
                            )
                        if j % 4 == 3 and pending:
                            pending.pop(0)()
                    for nn in range(2):
                        # copy psum out fast to free the accumulator slot, then
                        # normalize from SBUF off the critical path
                        ocp = rpool.tile([HD + 1, 512], F32, tag="ocp")
                        nc.vector.tensor_copy(ocp[:], oacc[nn][:])
                        rec = rpool.tile([1, 512], FR, tag="rec")
                        with nc.allow_low_precision(reason="f32r 1/Z, ~1e-4 rel"):
                            nc.vector.reciprocal(rec[:], ocp[HD : HD + 1, :])
                        bc = ps_m.tile([64, 512], F32, tag="m")
                        nc.tensor.matmul(bc[:], ones_r[:], rec[:], start=True, stop=True)
                        nc.vector.tensor_mul(
                            stack[t][r : r + 64, i0 + nn * 512 : i0 + (nn + 1) * 512],
                            ocp[0:HD, :],
                            bc[:],
                        )
                for nn in range(2):
                    for fb in range(D // P):
                        pending.append(
                            lambda fb=fb, n=2 * half + nn: proj_unit(fb, n)
                        )
            while pending:
                pending.pop(0)()

    nc.compile()
    return nc


def get_nc():
    global _CACHED
    if _CACHED is None:
        _CACHED = _build()
    return _CACHED


def make_in_maps(query, key, value, Wq, bq, Wk, bk, Wv, bv, Wo, bo):
    f32 = lambda a: np.ascontiguousarray(np.asarray(a, dtype=np.float32))
    in_maps = []
    for c in range(8):
        b, g = divmod(c, 4)
        gs = slice(F * g, F * (g + 1))
        in_maps.append(
            {
                "xq": f32(np.asarray(query)[b].T),
                "xk": f32(np.asarray(key)[b].T),
                "xv": f32(np.asarray(value)[b].T),
                "wq": f32(np.asarray(Wq)[gs, :].T),
                "wk": f32(np.asarray(Wk)[gs, :].T),
                "wv": f32(np.asarray(Wv)[gs, :].T),
                "wo": f32(np.asarray(Wo)[:, gs].T),
                "bq": f32(np.asarray(bq)[gs].reshape(F // P, P).T),
                "bk": f32(np.asarray(bk)[gs].reshape(F // P, P).T),
            }
        )
    return in_maps


def kernel(query, key, value, Wq, bq, Wk, bk, Wv, bv, Wo, bo):
    nc = get_nc()
    in_maps = make_in_maps(query, key, value, Wq, bq, Wk, bk, Wv, bv, Wo, bo)
    res = run_bass_kernel_spmd(nc, in_maps, core_ids=list(range(8)))
    bias_total = (
        np.asarray(bo, dtype=np.float64)
        + np.asarray(Wo, dtype=np.float64) @ np.asarray(bv, dtype=np.float64)
    ).astype(np.float32)
    outs = []
    for b in range(B):
        acc = np.zeros((D, S), np.float32)
        for g in range(G):
            acc += res.results[G * b + g]["ot"]
        outs.append(acc.T + bias_total[None, :])
    return np.stack(outs).astype(np.float32)


# revision 43
# speedup vs baseline: 102.1712x; 1.0249x over previous
"""Trainium2 Bass kernel for nn_Attention (B=2, S=2048, D=1024, H=16).

Sharding: 8 cores = 2 batches x 4 head-groups (4 heads each), Megatron-style:
column-parallel QKV projections, local attention, row-parallel output
projection; host reduces the 4 partial outputs per batch and adds biases.

Per-core dataflow (all matmuls in float32r: full-rate, ~1.5e-4 rel rounding):
  stage 1: qT/kT = W @ x.T (features on partitions, seq on free), written with
           head rows DUPLICATED into both partition halves so each head's two
           512-wide i-slices run as one row-packed matmul pair
           (tile_position (0,0)/(64,0) -> concurrent K=64 matmuls, 530ns/pair
           vs 2x395ns; per-matmul cost here is dominated by the serial
           LDWEIGHTS the toolchain emits with --enable-ldw-opt=false).
           v = x @ W.T (seq on partitions) with an appended ones column.
  stage 2 (per head, per 1024-wide i-slice): sT[j,i] = k_j.q_i -> one
           exp([128,1024]) on ACT -> e[j,i] (f32r); outT[d,i] accumulates
           vplus[j,d]*e[j,i] over j in PSUM, row 64 = sum_j e = Z (softmax
           denominator for free, no max-subtraction needed: raw scores are
           bounded ~+-20). Normalize via 1/Z broadcast with a K=1 ones-matmul.
  stage 3: finalT[f,i] = Wo_g.T @ stacked-heads (partial, host-reduced);
           projection units + second-half q-projections are interleaved into
           stage-2's loop where the PE would otherwise idle behind ACT.

Host folds the v-bias through attention (softmax rows sum to 1) into the
output bias: final_bias = bo + Wo @ bv.

Measured on trn2 (8-core SPMD, loop-slope method): ~365 us/run per core,
rel_l2 vs fp32 reference 5.6e-4.
"""

import numpy as np

import concourse.bass as bass
import concourse.mybir as mybir
import concourse.tile as tile
from concourse import bacc
from concourse.bass_utils import run_bass_kernel_spmd

B, S, D = 2, 2048, 1024
H, HD = 16, 64
G = 4              # head-groups == cores per batch
GH = H // G        # heads per core
F = GH * HD        # per-core projected features (256)
P = 128
KT = D // P        # 8 contraction tiles for the projections
NS = S // 512      # 4 seq tiles of 512
NC = S // P        # 16 seq chunks of 128
FR = mybir.dt.float32r
F32 = mybir.dt.float32
EXP = mybir.ActivationFunctionType.Exp

_CACHED = None


def _build(reps=None):
    import contextlib

    nc = bacc.Bacc("TRN2", target_bir_lowering=False, debug=False, num_devices=8)

    xq = nc.dram_tensor("xq", [D, S], FR, kind="ExternalInput").ap()
    xk = nc.dram_tensor("xk", [D, S], FR, kind="ExternalInput").ap()
    xv = nc.dram_tensor("xv", [D, S], FR, kind="ExternalInput").ap()
    wq = nc.dram_tensor("wq", [D, F], FR, kind="ExternalInput").ap()
    wk = nc.dram_tensor("wk", [D, F], FR, kind="ExternalInput").ap()
    wv = nc.dram_tensor("wv", [D, F], FR, kind="ExternalInput").ap()
    wo = nc.dram_tensor("wo", [F, D], FR, kind="ExternalInput").ap()
    bq = nc.dram_tensor("bq", [P, F // P], F32, kind="ExternalInput").ap()
    bk = nc.dram_tensor("bk", [P, F // P], F32, kind="ExternalInput").ap()
    ot = nc.dram_tensor("ot", [D, S], F32, kind="ExternalOutput").ap()

    xq_r = xq.rearrange("(ko p) s -> p ko s", p=P)
    xk_r = xk.rearrange("(ko p) s -> p ko s", p=P)
    xv_r = xv.rearrange("(ko p) s -> p ko s", p=P)
    wq_r = wq.rearrange("(ko p) f -> p ko f", p=P)
    wk_r = wk.rearrange("(ko p) f -> p ko f", p=P)
    wv_r = wv.rearrange("(ko p) f -> p ko f", p=P)
    wo_r = wo.rearrange("(ko p) f -> p ko f", p=P)
    ot_r = ot.rearrange("(fo p) s -> p fo s", p=P)

    with tile.TileContext(nc) as tc:
        with (
            tc.tile_pool(name="wpool", bufs=1) as wpool,
            tc.tile_pool(name="xpool", bufs=2) as xpool,
            tc.tile_pool(name="apool", bufs=1) as apool,
            tc.tile_pool(name="epool", bufs=4) as epool,
            tc.tile_pool(name="rpool", bufs=3) as rpool,
            tc.tile_pool(name="opool", bufs=4) as opool,
            tc.tile_pool(name="ps_s", bufs=2, space="PSUM") as ps_s,
            tc.tile_pool(name="ps_o", bufs=2, space="PSUM") as ps_o,
            tc.tile_pool(name="ps_m", bufs=2, space="PSUM") as ps_m,
        ):
          with (
              tc.For_i(0, reps, 1, hint_engines=(mybir.EngineType.PE, mybir.EngineType.DVE, mybir.EngineType.Activation, mybir.EngineType.SP))
              if reps
              else contextlib.nullcontext()
          ):
            # ---- constants / weights (DMA-ordered by first use) ----
            wq_sb = wpool.tile([P, KT, F], FR)
            wk_sb = wpool.tile([P, KT, F], FR)
            wv_sb = wpool.tile([P, KT, F], FR)
            wo_sb = wpool.tile([P, F // P, D], FR)
            bq_sb = wpool.tile([P, F // P], F32)
            bk_sb = wpool.tile([P, F // P], F32)
            nc.sync.dma_start(wk_sb[:], wk_r)
            nc.sync.dma_start(bk_sb[:], bk)

            ones_f = wpool.tile([1, 64], F32)
            nc.vector.memset(ones_f[:], 1.0)
            ones_r = wpool.tile([1, 64], FR)
            nc.vector.tensor_copy(ones_r[:], ones_f[:])
            onesp_f = wpool.tile([P, 1], F32)
            nc.vector.memset(onesp_f[:], 1.0)
            onesp_r = wpool.tile([P, 1], FR)
            nc.vector.tensor_copy(onesp_r[:], onesp_f[:])

            # ---- persistent activations ----
            # per-head q/k tiles with rows duplicated into both partition
            # halves, so a head's two i-halves can run as one row-packed
            # matmul pair (tile_position (0,0) / (64,0))
            qTd = [
                apool.tile([P, S], FR, name=f"qTd{h}", tag=f"qTd{h}") for h in range(GH)
            ]
            kTd = [
                apool.tile([P, S], FR, name=f"kTd{h}", tag=f"kTd{h}") for h in range(GH)
            ]
            stack = [
                apool.tile([P, S], FR, name=f"stack{t}", tag=f"stack{t}")
                for t in range(2)
            ]
            v_sb = apool.tile([P, NC, GH, HD + 1], FR, name="v_sb")

            # ---- stage 1: k and v projections (then q) ----
            def proj_qk_t(dst, w_sb, b_sb, x_n, n, t):
                ps = ps_m.tile([P, 512], F32, tag="m", name="ps")
                for k in range(KT):
                    nc.tensor.matmul(
                        ps[:],
                        w_sb[:, k, t * P : (t + 1) * P],
                        x_n[:, k, :],
                        start=(k == 0),
                        stop=(k == KT - 1),
                    )
                for hh in range(2):
                    h = 2 * t + hh
                    r = hh * 64
                    for dup in range(2):
                        nc.vector.tensor_scalar_add(
                            dst[h][dup * 64 : dup * 64 + 64, n * 512 : (n + 1) * 512],
                            ps[r : r + 64, :],
                            b_sb[r : r + 64, t : t + 1],
                        )

            def proj_qk(dst, w_sb, b_sb, x_n, n):
                for t in range(2):
                    proj_qk_t(dst, w_sb, b_sb, x_n, n, t)

            def _xdma(dst, src_r, n):
                for k in range(KT):
                    nc.sync.dma_start(
                        dst[:, k], src_r[:, k, n * 512 : (n + 1) * 512]
                    )

            def kproj(n, ts=(0, 1)):
                xk_n = xpool.tile([P, KT, 512], FR, tag="xin", name="xk_n")
                _xdma(xk_n, xk_r, n)
                for t in ts:
                    proj_qk_t(kTd, wk_sb, bk_sb, xk_n, n, t)

            def qproj(n, ts=(0, 1)):
                xq_n = xpool.tile([P, KT, 512], FR, tag="xin", name="xq_n")
                _xdma(xq_n, xq_r, n)
                for t in ts:
                    proj_qk_t(qTd, wq_sb, bq_sb, xq_n, n, t)

            def vproj(n):
                xv_n = xpool.tile([P, KT, 512], FR, tag="xin", name="xv_n")
                _xdma(xv_n, xv_r, n)
                for c in range(4):
                    psv = ps_m.tile([P, F], F32, tag="m")
                    for k in range(KT):
                        nc.tensor.matmul(
                            psv[:],
                            xv_n[:, k, c * P : (c + 1) * P],
                            wv_sb[:, k, :],
                            start=(k == 0),
                            stop=(k == KT - 1),
                        )
                    ch = n * 4 + c
                    nc.vector.tensor_copy(
                        v_sb[:, ch, :, 0:HD],
                        psv.rearrange("p (h e) -> p h e", e=HD),
                    )
                    nc.vector.tensor_copy(
                        v_sb[:, ch, :, HD : HD + 1],
                        onesp_r[:, 0:1, None].to_broadcast((P, GH, 1)),
                    )

            # order: the minimum attention-head-0/1 needs runs up front
            # (t=0 halves of k and first-half q, all of v); the rest is
            # deferred into the attention loop's interleave slots where the
            # PE would otherwise idle while ACT grinds exps
            kproj(0)
            nc.sync.dma_start(wq_sb[:], wq_r)
            nc.sync.dma_start(bq_sb[:], bq)
            qproj(0)
            qproj(1)
            nc.sync.dma_start(wv_sb[:], wv_r)
            vproj(0)
            kproj(1)
            vproj(1)
            kproj(2)
            vproj(2)
            kproj(3)
            vproj(3)
            nc.sync.dma_start(wo_sb[:], wo_r)

            # ---- stage 2+3: attention (half-outer; output-projection units of
            # a finished i-half are interleaved between the next half's heads) ----
            def proj_unit(fb, n):
                pf = ps_m.tile([P, 512], F32, tag="m")
                for kk in range(F // P):
                    nc.tensor.matmul(
                        pf[:],
                        wo_sb[:, kk, fb * P : (fb + 1) * P],
                        stack[kk][:, n * 512 : (n + 1) * 512],
                        start=(kk == 0),
                        stop=(kk == F // P - 1),
                    )
                ob = opool.tile([P, 512], F32, tag="ob")
                nc.vector.tensor_copy(ob[:], pf[:])
                nc.sync.dma_start(ot_r[:, fb, n * 512 : (n + 1) * 512], ob[:])

            pending = [lambda n=n: qproj(n) for n in (2, 3)]
            for half in range(2):
                i0 = half * 1024
                for h in range(GH):
                    t, r = h // 2, (h % 2) * 64
                    oacc = [
                        ps_o.tile([HD + 1, 512], F32, tag="oacc", name=f"oacc{_nn}")
                        for _nn in range(2)
                    ]
                    for j in range(NC):
                        ss = ps_s.tile([P, 1024], F32, tag="ss")
                        for nn in range(2):
                            rb = nn * 64
                            nc.tensor.matmul(
                                ss[:, nn * 512 : (nn + 1) * 512],
                                kTd[h][rb : rb + 64, j * P : (j + 1) * P],
                                qTd[h][rb : rb + 64, i0 + nn * 512 : i0 + (nn + 1) * 512],
                                start=True,
                                stop=True,
                                tile_position=(rb, 0),
                            )
                        eb = epool.tile([P, 1024], FR, tag="eb")
                        nc.scalar.activation(eb[:], ss[:], EXP)
                        for nn in range(2):
                            nc.tensor.matmul(
                                oacc[nn][:],
                                v_sb[:, j, h, :],
                                eb[:, nn * 512 : (nn + 1) * 512],
                                start=(j == 0),
                                stop=(j == NC - 1),
                            )
                        if j % 4 == 3 and pending:
                            pending.pop(0)()
                    for nn in range(2):
                        # copy psum out fast to free the accumulator slot, then
                        # normalize from SBUF off the critical path
                        ocp = rpool.tile([HD + 1, 512], F32, tag="ocp")
                        nc.vector.tensor_copy(ocp[:], oacc[nn][:])
                        rec = rpool.tile([1, 512], FR, tag="rec")
                        with nc.allow_low_precision(reason="f32r 1/Z, ~1e-4 rel"):
                            nc.vector.reciprocal(rec[:], ocp[HD : HD + 1, :])
                        bc = ps_m.tile([64, 512], F32, tag="m")
                        nc.tensor.matmul(bc[:], ones_r[:], rec[:], start=True, stop=True)
                        nc.vector.tensor_mul(
                            stack[t][r : r + 64, i0 + nn * 512 : i0 + (nn + 1) * 512],
                            ocp[0:HD, :],
                            bc[:],
                        )
                for nn in range(2):
                    for fb in range(D // P):
                        pending.append(
                            lambda fb=fb, n=2 * half + nn: proj_unit(fb, n)
                        )
            while pending:
                pending.pop(0)()

    nc.compile()
    return nc


def get_nc():
    global _CACHED
    if _CACHED is None:
        _CACHED = _build()
    return _CACHED


def make_in_maps(query, key, value, Wq, bq, Wk, bk, Wv, bv, Wo, bo):
    f32 = lambda a: np.ascontiguousarray(np.asarray(a, dtype=np.float32))
    in_maps = []
    for c in range(8):
        b, g = divmod(c, 4)
        gs = slice(F * g, F * (g + 1))
        in_maps.append(
            {
                "xq": f32(np.asarray(query)[b].T),
                "xk": f32(np.asarray(key)[b].T),
                "xv": f32(np.asarray(value)[b].T),
                "wq": f32(np.asarray(Wq)[gs, :].T),
                "wk": f32(np.asarray(Wk)[gs, :].T),
                "wv": f32(np.asarray(Wv)[gs, :].T),
                "wo": f32(np.asarray(Wo)[:, gs].T),
                "bq": f32(np.asarray(bq)[gs].reshape(F // P, P).T),
                "bk": f32(np.asarray(bk)[gs].reshape(F // P, P).T),
            }
        )
    return in_maps


def kernel(query, key, value, Wq, bq, Wk, bk, Wv, bv, Wo, bo):
    nc = get_nc()
    in_maps = make_in_maps(query, key, value, Wq, bq, Wk, bk, Wv, bv, Wo, bo)
    res = run_bass_kernel_spmd(nc, in_maps, core_ids=list(range(8)))
    bias_total = (
        np.asarray(bo, dtype=np.float64)
        + np.asarray(Wo, dtype=np.float64) @ np.asarray(bv, dtype=np.float64)
    ).astype(np.float32)
    outs = []
    for b in range(B):
        acc = np.zeros((D, S), np.float32)
        for g in range(G):
            acc += res.results[G * b + g]["ot"]
        outs.append(acc.T + bias_total[None, :])
    return np.stack(outs).astype(np.float32)
